# revision 1
# baseline (speedup 1.0000x reference)
# Chunked causal self-attention (Megalodon-style, chunk=2048) on 8 Trainium2
# NeuronCores via Bass/Tile.
#
# Problem (hardcoded): q,k,v (2, 4096, 16, 128) fp32, RoPE(10000) on q,k,
# per-chunk causal softmax(QK^T)V with scale 1.0.
#
# Sharding: 64 independent (batch, chunk, head) attention units of size
# (2048 x 2048 x 128); 8 units per core (4 (b,h) pairs x 2 chunks).
#
# Per-unit device pipeline:
#   DMA q,k (fp32, pre-transposed host layout) -> RoPE on DVE/ACT (3 TT passes)
#   -> PE transpose to [d, pos] (f32r)  -> S^T = K^T.T @ Q^T  (f32r matmuls)
#   -> +mask matmul on diagonal 128-blocks (bf16) -> ACT exp -> probs bf16
#   -> ones-matmul (denominators, replicated across partitions)
#   -> O^T = V.T-free accumulate (lhsT=V bf16, rhs=probs bf16)
#   -> recip_approx(denom) * O^T on DVE -> DMA out O^T (host transposes back).
import numpy as np
import ml_dtypes

B, T, H, DH, DV = 2, 4096, 16, 128, 128
CHUNK = 2048
NB = CHUNK // 128          # 16 key blocks per chunk
N_CORES = 8
UNITS = 8                  # (b,h) pairs per core * 2 chunks
BH_PER_CORE = (B * H) // N_CORES   # 4
ROPE_BASE = 10000.0
NEG = -1e30
QH = 1024                  # q-half width processed per pass (PSUM budget)

_RUNTIME = {}


def _build_program(reps=1):
    import concourse.tile as tile
    import concourse.mybir as mybir
    from concourse import bacc

    f32 = mybir.dt.float32
    f32r = mybir.dt.float32r
    bf16 = mybir.dt.bfloat16
    Exp = mybir.ActivationFunctionType.Exp

    nc = bacc.Bacc("TRN2", target_bir_lowering=False, debug=False,
                   num_devices=N_CORES)

    qc = nc.dram_tensor("qc", [UNITS, 128, CHUNK], f32, kind="ExternalInput").ap()
    kc = nc.dram_tensor("kc", [UNITS, 128, CHUNK], f32, kind="ExternalInput").ap()
    vc = nc.dram_tensor("vc", [UNITS, 128, CHUNK], bf16, kind="ExternalInput").ap()
    cosf = nc.dram_tensor("cosf", [2, 128, CHUNK], f32, kind="ExternalInput").ap()
    sinf = nc.dram_tensor("sinf", [2, 128, CHUNK], f32, kind="ExternalInput").ap()
    ident = nc.dram_tensor("ident", [128, 128], f32, kind="ExternalInput").ap()
    ident16 = nc.dram_tensor("ident16", [128, 128], bf16, kind="ExternalInput").ap()
    mask16 = nc.dram_tensor("mask16", [128, 128], bf16, kind="ExternalInput").ap()
    outT = nc.dram_tensor("outT", [UNITS, 128, CHUNK], f32, kind="ExternalOutput").ap()

    with tile.TileContext(nc) as tc:
        with tc.tile_pool(name="const", bufs=1) as cpool, \
             tc.tile_pool(name="work", bufs=2) as wpool, \
             tc.tile_pool(name="scratch", bufs=1) as spool, \
             tc.tile_pool(name="qkT", bufs=4) as tpool, \
             tc.tile_pool(name="probs", bufs=8) as ppool, \
             tc.tile_pool(name="psum", bufs=2, space="PSUM") as pspool, \
             tc.tile_pool(name="psumO", bufs=1, space="PSUM") as popool, \
             tc.tile_pool(name="psumD", bufs=1, space="PSUM") as pdpool:

            tcos = cpool.tile([128, 2 * CHUNK], f32, tag="tcos")
            tsin = cpool.tile([128, 2 * CHUNK], f32, tag="tsin")
            tid = cpool.tile([128, 128], f32, tag="tid")
            tidr = cpool.tile([128, 128], f32r, tag="tidr")
            tid16 = cpool.tile([128, 128], bf16, tag="tid16")
            tmask = cpool.tile([128, 128], bf16, tag="tmask")
            tones = cpool.tile([128, 128], bf16, tag="tones")
            for ch in range(2):
                nc.gpsimd.dma_start(out=tcos[:, ch * CHUNK:(ch + 1) * CHUNK], in_=cosf[ch])
                nc.gpsimd.dma_start(out=tsin[:, ch * CHUNK:(ch + 1) * CHUNK], in_=sinf[ch])
            nc.gpsimd.dma_start(out=tid[:], in_=ident[:])
            nc.gpsimd.dma_start(out=tid16[:], in_=ident16[:])
            nc.gpsimd.dma_start(out=tmask[:], in_=mask16[:])
            nc.gpsimd.memset(tones[:], 1.0)
            nc.vector.tensor_copy(tidr[:], tid[:])

            def load_rope(u):
                """DMA q,k,v of unit u + RoPE on DVE. Returns rope outputs
                (t1 tiles) + v tile."""
                ch = u % 2
                cosv = tcos[:, ch * CHUNK:(ch + 1) * CHUNK]
                sinv = tsin[:, ch * CHUNK:(ch + 1) * CHUNK]
                s4 = sinv.rearrange("p (b two d) -> p b two d", two=2, d=64)
                t1s = []
                for src in (qc, kc):
                    raw = wpool.tile([128, CHUNK], f32, tag="raw")
                    t1 = wpool.tile([128, CHUNK], f32r, tag="t1")
                    t2 = spool.tile([128, CHUNK], f32, tag="t2")
                    nc.sync.dma_start(out=raw[:], in_=src[u])
                    r4 = raw[:].rearrange("p (b two d) -> p b two d", two=2, d=64)
                    o4 = t2[:].rearrange("p (b two d) -> p b two d", two=2, d=64)
                    nc.any.tensor_mul(t1[:], raw[:], cosv)
                    nc.any.tensor_mul(o4[:, :, 0, :], r4[:, :, 1, :], s4[:, :, 0, :])
                    nc.any.tensor_mul(o4[:, :, 1, :], r4[:, :, 0, :], s4[:, :, 1, :])
                    nc.any.tensor_add(t1[:], t1[:], t2[:])
                    t1s.append(t1)
                tv = wpool.tile([128, CHUNK], bf16, tag="tv")
                nc.sync.dma_start(out=tv[:], in_=vc[u])
                return t1s[0], t1s[1], tv

            def transposes(t1q, t1k, psum_pool, ptag):
                """PE-transpose rope outputs into [d, pos] f32r SBUF tiles."""
                outs = []
                for t1 in (t1q, t1k):
                    dstT = tpool.tile([128, CHUNK], f32r, tag="tT")
                    for half in range(2):
                        pst = psum_pool.tile([128, QH], f32r, tag=ptag)
                        for blk in range(8):
                            g = half * 8 + blk
                            nc.tensor.transpose(
                                pst[:, blk * 128:(blk + 1) * 128],
                                t1[:, g * 128:(g + 1) * 128], tidr[:])
                        nc.scalar.copy(dstT[:, half * QH:(half + 1) * QH],
                                       pst[:])
                    outs.append(dstT)
                return outs[0], outs[1]

            def attention_half(u, hf, tqt, tkt, tv, after_first_row=None):
                jmax = 8 * hf + 7
                psO = popool.tile([128, QH], f32, tag="psO")
                psD = pdpool.tile([128, QH], f32, tag="psD")
                pending = []

                def emit_consumers(j, oj, probs):
                    # psD chunks first, then psO chunks: keeps the stationary
                    # operand (ones / V_j) constant across consecutive matmuls
                    for ps, lhsT in ((psD, tones[:]),
                                     (psO, tv[:, j * 128:(j + 1) * 128])):
                        for s in (0, 1):
                            lo, hi = max(oj, 512 * s), 512 * (s + 1)
                            if lo >= hi:
                                continue
                            last = (j == min(jmax, 8 * hf + 4 * s + 3))
                            nc.tensor.matmul(ps[:, lo:hi], lhsT=lhsT,
                                             rhs=probs[:, lo:hi],
                                             start=(j == 0), stop=last)

                for j in range(jmax + 1):
                    oj = max(0, 128 * j - QH * hf)
                    diag = (j >= 8 * hf)
                    psS = pspool.tile([128, QH], f32, tag="psS")
                    for s in (0, 1):
                        lo, hi = max(oj, 512 * s), 512 * (s + 1)
                        if lo >= hi:
                            continue
                        in_diag_bank = diag and (oj >= 512 * s) and (oj < hi)
                        nc.tensor.matmul(
                            psS[:, lo:hi],
                            lhsT=tkt[:, j * 128:(j + 1) * 128],
                            rhs=tqt[:, hf * QH + lo: hf * QH + hi],
                            start=True, stop=not in_diag_bank)
                        if in_diag_bank:
                            nc.tensor.matmul(
                                psS[:, oj:oj + 128], lhsT=tid16[:],
                                rhs=tmask[:], start=False, stop=True,
                                skip_group_check=True)
                    probs = ppool.tile([128, QH], bf16, tag="probs")
                    nc.scalar.activation(probs[:, oj:QH], psS[:, oj:QH], Exp)
                    pending.append((j, oj, probs))
                    if len(pending) > 3:
                        emit_consumers(*pending.pop(0))
                    if j == 0 and after_first_row is not None:
                        after_first_row()
                while pending:
                    emit_consumers(*pending.pop(0))

                rec = wpool.tile([128, QH], f32, tag="rec")
                osb = wpool.tile([128, QH], f32, tag="osb")
                for s in (0, 1):
                    sl = slice(512 * s, 512 * (s + 1))
                    nc.vector.reciprocal_approx_fast(out=rec[:, sl], in_=psD[:, sl])
                    nc.any.tensor_mul(osb[:, sl], psO[:, sl], rec[:, sl])
                    nc.sync.dma_start(
                        out=outT[u, :, hf * QH + 512 * s: hf * QH + 512 * (s + 1)],
                        in_=osb[:, sl])

            r = load_rope(0)
            cur = transposes(r[0], r[1], pspool, "psS") + (r[2],)
            for _rep in range(reps):
                for u in range(UNITS):
                    # prefetch next unit (wrapping into the next rep):
                    # DMA + rope before this unit's attention
                    has_next = (u + 1 < UNITS) or (_rep + 1 < reps)
                    if has_next:
                        nxt = load_rope((u + 1) % UNITS)
                    attention_half(u, 0, cur[0], cur[1], cur[2])
                    # transpose next unit's rope output inside half 1, after
                    # its first S row (borrows the psD slot, which frees once
                    # half 0's reciprocal has read it)
                    holder = {}
                    hook = None
                    if has_next:
                        def hook(nxt=nxt, holder=holder):
                            holder["T"] = transposes(nxt[0], nxt[1], pdpool, "psD")
                    attention_half(u, 1, cur[0], cur[1], cur[2],
                                   after_first_row=hook)
                    if has_next:
                        nxt_T = holder["T"]
                        cur = (nxt_T[0], nxt_T[1], nxt[2])
    nc.compile()
    return nc


def _make_runner(nc):
    """Cached PJRT runner (clone of bass2jax.run_bass_via_pjrt multi-core
    path, but keeping the jitted callable so repeat calls don't recompile)."""
    import jax
    import concourse.mybir as mybir
    from concourse import bass2jax
    from jax.sharding import Mesh, PartitionSpec
    from jax.experimental.shard_map import shard_map

    bass2jax.install_neuronx_cc_hook()

    partition_name = (nc.partition_id_tensor.name
                      if nc.partition_id_tensor else None)
    in_names, out_names, out_avals, zero_outs = [], [], [], []
    for alloc in nc.m.functions[0].allocations:
        if not isinstance(alloc, mybir.MemoryLocationSet):
            continue
        name = alloc.memorylocations[0].name
        if alloc.kind == "ExternalInput":
            if name != partition_name:
                in_names.append(name)
        elif alloc.kind == "ExternalOutput":
            shape = tuple(alloc.tensor_shape)
            dtype = mybir.dt.np(alloc.dtype)
            out_names.append(name)
            out_avals.append(jax.core.ShapedArray(shape, dtype))
            zero_outs.append(np.zeros(shape, dtype))
    n_params = len(in_names)
    n_outs = len(out_avals)
    all_names = in_names + out_names
    if partition_name is not None:
        all_names = all_names + [partition_name]
    donate = tuple(range(n_params, n_params + n_outs))

    def _body(*args):
        operands = list(args)
        if partition_name is not None:
            operands.append(bass2jax.partition_id_tensor())
        outs = bass2jax._bass_exec_p.bind(
            *operands, out_avals=tuple(out_avals), in_names=tuple(all_names),
            out_names=tuple(out_names), lowering_input_output_aliases=(),
            sim_require_finite=True, sim_require_nnan=True, nc=nc)
        return tuple(outs)

    devices = jax.devices()[:N_CORES]
    mesh = Mesh(np.asarray(devices), ("core",))
    sharded = jax.jit(
        shard_map(_body, mesh=mesh,
                  in_specs=(PartitionSpec("core"),) * (n_params + n_outs),
                  out_specs=(PartitionSpec("core"),) * n_outs,
                  check_rep=False),
        donate_argnums=donate, keep_unused=True)

    def run(in_maps):
        concat_in = [np.concatenate([m[name] for m in in_maps], axis=0)
                     for name in in_names]
        concat_zero = [np.concatenate([z] * N_CORES, axis=0) for z in zero_outs]
        outs = sharded(*concat_in, *concat_zero)
        outs = [np.asarray(o) for o in outs]
        res = []
        for c in range(N_CORES):
            d = {}
            for i, name in enumerate(out_names):
                per = outs[i].shape[0] // N_CORES
                d[name] = outs[i][c * per:(c + 1) * per]
            res.append(d)
        return res

    return run


def _rope_tables(start_index):
    half = DH // 2
    inv_freq = np.exp(np.arange(half, dtype=np.float64) *
                      (-(np.log(ROPE_BASE) / half)))
    pos = np.arange(T, dtype=np.float64) + float(start_index)
    ang = pos[:, None] * inv_freq[None, :]          # (T, 64)
    cos = np.cos(ang)
    sin = np.sin(ang)
    cosfull = np.concatenate([cos, cos], axis=1)    # (T, 128)
    sinfull = np.concatenate([-sin, sin], axis=1)
    # (T,128) -> (2, 16, 128, 128)[c, pb, p, d] -> (2, 128, 16*128)
    def lay(x):
        x = x.reshape(2, NB, 128, DH).transpose(0, 2, 1, 3).reshape(2, 128, CHUNK)
        return np.ascontiguousarray(x, dtype=np.float32)
    return lay(cosfull), lay(sinfull)


def _shard_inputs(q, k, v, start_index):
    q = np.asarray(q, dtype=np.float32)
    k = np.asarray(k, dtype=np.float32)
    v = np.asarray(v, dtype=np.float32)
    cosf, sinf = _rope_tables(start_index)
    ident = np.eye(128, dtype=np.float32)
    i = np.arange(128)
    mask16 = np.where(i[:, None] <= i[None, :], 0.0, NEG).astype(ml_dtypes.bfloat16)

    # layout per unit: [p, blk*128+d] with pos = blk*128 + p
    def lay(x):  # (2048, 128) -> (128, 2048)
        return x.reshape(NB, 128, DH).transpose(1, 0, 2).reshape(128, CHUNK)

    in_maps = []
    for c in range(N_CORES):
        qu = np.empty((UNITS, 128, CHUNK), np.float32)
        ku = np.empty((UNITS, 128, CHUNK), np.float32)
        vu = np.empty((UNITS, 128, CHUNK), ml_dtypes.bfloat16)
        for ubh in range(BH_PER_CORE):
            bh = c * BH_PER_CORE + ubh
            b, h = bh // H, bh % H
            for ch in range(2):
                u = ubh * 2 + ch
                sl = slice(ch * CHUNK, (ch + 1) * CHUNK)
                qu[u] = lay(q[b, sl, h, :])
                ku[u] = lay(k[b, sl, h, :])
                vu[u] = lay(v[b, sl, h, :]).astype(ml_dtypes.bfloat16)
        in_maps.append({"qc": qu, "kc": ku, "vc": vu, "cosf": cosf,
                        "sinf": sinf, "ident": ident,
                        "ident16": ident.astype(ml_dtypes.bfloat16),
                        "mask16": mask16})
    return in_maps


def _gather_output(results):
    out = np.empty((B, T, H, DV), np.float32)
    for c in range(N_CORES):
        oT = results[c]["outT"]        # (UNITS, 128 dv, 2048 q)
        for ubh in range(BH_PER_CORE):
            bh = c * BH_PER_CORE + ubh
            b, h = bh // H, bh % H
            for ch in range(2):
                u = ubh * 2 + ch
                out[b, ch * CHUNK:(ch + 1) * CHUNK, h, :] = oT[u].T
    return out


def get_runtime(reps=1):
    if reps not in _RUNTIME:
        nc = _build_program(reps)
        _RUNTIME[reps] = _make_runner(nc)
    return _RUNTIME[reps]


def kernel(q, k, v, start_index):
    run = get_runtime()
    in_maps = _shard_inputs(q, k, v, start_index)
    results = run(in_maps)
    return _gather_output(results)


if __name__ == "__main__":
    rng = np.random.default_rng(0)
    q = rng.standard_normal((B, T, H, DH)).astype(np.float32)
    k = rng.standard_normal((B, T, H, DH)).astype(np.float32)
    v = rng.standard_normal((B, T, H, DV)).astype(np.float32)
    out = kernel(q, k, v, 0)
    print("out", out.shape, out.dtype, np.abs(out).max())



# revision 3
# speedup vs baseline: 1.5425x; 1.5425x over previous
# Chunked causal self-attention (Megalodon-style, chunk=2048) on 8 Trainium2
# NeuronCores via Bass/Tile — v2.
#
# Problem (hardcoded): q,k,v (2, 4096, 16, 128) fp32, RoPE(10000) on q,k,
# per-chunk causal softmax(QK^T)V with scale 1.0.
#
# Sharding: 64 independent (batch, chunk, head) attention units of size
# (2048 x 2048 x 128); 8 units per core (4 (b,h) pairs x 2 chunks).
#
# v2 design (vs v1): engine-balanced around the two irreducible loads —
# PE matmul (S + PV, 1 cyc/col) and ACT exp (only engine with Exp).
#   * q,k arrive fp16 in [d, pos] layout PLUS partition-swapped copies
#     (rows rotated by 64), so RoPE runs on DVE lane-aligned in fp16:
#     rq = q*cosF + qswap*sinF  (3 ops, 2-byte datapath)
#     -> NO PE transposes, NO PSUM->SBUF copies on ACT.
#   * S^T = K^T.T @ Q^T directly from the fp16 rope outputs (fp16 matmul,
#     1 cyc/col, exact-enough: rel err ~0.013 vs 2e-2 gate).
#   * softmax denominators: DVE accumulates probs tiles in bf16 into two
#     interleaved accumulators; ONE final ones-matmul pair per half
#     reduces over partitions (kills the per-j ones-matmul that used to
#     cost PE as much as PV).
#   * mask on diagonal 128-blocks stays a bf16 matmul (cheap, proven).
#   * exp on ACT -> probs bf16; reciprocal+normalize on DVE; DMA out O^T.
import numpy as np
import ml_dtypes

B, T, H, DH, DV = 2, 4096, 16, 128, 128
CHUNK = 2048
NB = CHUNK // 128          # 16 key blocks per chunk
N_CORES = 8
UNITS = 8                  # (b,h) pairs per core * 2 chunks
BH_PER_CORE = (B * H) // N_CORES   # 4
ROPE_BASE = 10000.0
NEG = -1e30
QH = 1024                  # q-half width processed per pass (PSUM budget)

_RUNTIME = {}

import os as _os
ROPE_ADD_POOL = int(_os.environ.get("ROPE_ADD_POOL", "0"))
H0_ACC_POOL = int(_os.environ.get("H0_ACC_POOL", "0"))
PEND_DEPTH = int(_os.environ.get("PEND_DEPTH", "4"))
PE_DENOM_N = int(_os.environ.get("PE_DENOM_N", "3"))


def _build_program(reps=1):
    import concourse.tile as tile
    import concourse.mybir as mybir
    from concourse import bacc

    f32 = mybir.dt.float32
    f16 = mybir.dt.float16
    bf16 = mybir.dt.bfloat16
    Exp = mybir.ActivationFunctionType.Exp

    nc = bacc.Bacc("TRN2", target_bir_lowering=False, debug=False,
                   num_devices=N_CORES)

    qc = nc.dram_tensor("qc", [UNITS, 128, CHUNK], f16, kind="ExternalInput").ap()
    qs = nc.dram_tensor("qs", [UNITS, 128, CHUNK], f16, kind="ExternalInput").ap()
    kc = nc.dram_tensor("kc", [UNITS, 128, CHUNK], f16, kind="ExternalInput").ap()
    ks = nc.dram_tensor("ks", [UNITS, 128, CHUNK], f16, kind="ExternalInput").ap()
    vc = nc.dram_tensor("vc", [UNITS, 128, CHUNK], bf16, kind="ExternalInput").ap()
    cosf = nc.dram_tensor("cosf", [2, 128, CHUNK], f16, kind="ExternalInput").ap()
    sinf = nc.dram_tensor("sinf", [2, 128, CHUNK], f16, kind="ExternalInput").ap()
    ident16 = nc.dram_tensor("ident16", [128, 128], bf16, kind="ExternalInput").ap()
    mask16 = nc.dram_tensor("mask16", [128, 128], bf16, kind="ExternalInput").ap()
    outT = nc.dram_tensor("outT", [UNITS, 128, CHUNK], f32, kind="ExternalOutput").ap()

    with tile.TileContext(nc) as tc:
        with tc.tile_pool(name="const", bufs=1) as cpool, \
             tc.tile_pool(name="raw", bufs=3) as rpool, \
             tc.tile_pool(name="rope", bufs=2) as qpool, \
             tc.tile_pool(name="work", bufs=2) as wpool, \
             tc.tile_pool(name="accs", bufs=2) as apool, \
             tc.tile_pool(name="probs", bufs=8) as ppool, \
             tc.tile_pool(name="psum", bufs=2, space="PSUM") as pspool, \
             tc.tile_pool(name="psumO", bufs=1, space="PSUM") as popool, \
             tc.tile_pool(name="psumD", bufs=1, space="PSUM") as pdpool:

            tcos = cpool.tile([128, 2 * CHUNK], f16, tag="tcos")
            tsin = cpool.tile([128, 2 * CHUNK], f16, tag="tsin")
            tid16 = cpool.tile([128, 128], bf16, tag="tid16")
            tmask = cpool.tile([128, 128], bf16, tag="tmask")
            tones = cpool.tile([128, 128], bf16, tag="tones")
            for ch in range(2):
                nc.gpsimd.dma_start(out=tcos[:, ch * CHUNK:(ch + 1) * CHUNK], in_=cosf[ch])
                nc.gpsimd.dma_start(out=tsin[:, ch * CHUNK:(ch + 1) * CHUNK], in_=sinf[ch])
            nc.gpsimd.dma_start(out=tid16[:], in_=ident16[:])
            nc.gpsimd.dma_start(out=tmask[:], in_=mask16[:])
            nc.gpsimd.memset(tones[:], 1.0)

            def load(u):
                """DMA q,k (+swapped) and v of unit u. Returns raw tiles."""
                tq = rpool.tile([128, CHUNK], f16, tag="tq")
                tqs = rpool.tile([128, CHUNK], f16, tag="tqs")
                tk = rpool.tile([128, CHUNK], f16, tag="tk")
                tks = rpool.tile([128, CHUNK], f16, tag="tks")
                tv = rpool.tile([128, CHUNK], bf16, tag="tv")
                nc.sync.dma_start(out=tq[:], in_=qc[u])
                nc.sync.dma_start(out=tqs[:], in_=qs[u])
                nc.sync.dma_start(out=tk[:], in_=kc[u])
                nc.sync.dma_start(out=tks[:], in_=ks[u])
                nc.sync.dma_start(out=tv[:], in_=vc[u])
                return tq, tqs, tk, tks, tv

            def rope(u, raw):
                """RoPE in fp16: r = x*cosF + xswap*sinF. Muls on DVE, the
                final adds on GPSIMD (SBUF-only, Pool is otherwise idle)."""
                tq, tqs, tk, tks, tv = raw
                ch = u % 2
                cosv = tcos[:, ch * CHUNK:(ch + 1) * CHUNK]
                sinv = tsin[:, ch * CHUNK:(ch + 1) * CHUNK]
                outs = []
                for x, xs, tag in ((tq, tqs, "rq"), (tk, tks, "rk")):
                    r = qpool.tile([128, CHUNK], f16, tag=tag)
                    nc.vector.tensor_mul(r[:], x[:], cosv)
                    nc.vector.tensor_mul(xs[:], xs[:], sinv)
                    eng = nc.gpsimd if ROPE_ADD_POOL else nc.vector
                    eng.tensor_add(r[:], r[:], xs[:])
                    outs.append(r)
                return outs[0], outs[1], tv

            def attention_half(u, hf, rq, rk, tv, hook=None,
                               finish_prev=None):
                jmax = 8 * hf + 7
                psO = popool.tile([128, QH], f32, tag="psO")
                acc = [apool.tile([128, QH], bf16, tag=f"acc{i}",
                                  name=f"acc{i}") for i in (0, 1)]
                # h1 denominator js handled directly on PE (ones-matmul into
                # the psD accumulation group) instead of DVE adds
                pe_js = set(range(2, 2 + PE_DENOM_N)) if hf == 1 else set()
                psD = [None]
                pending = []

                def denom_mm(tile, first, last):
                    if psD[0] is None:
                        psD[0] = pdpool.tile([128, QH], f32, tag="psD",
                                             name="psD")
                    for s in (0, 1):
                        sl = slice(512 * s, 512 * (s + 1))
                        nc.tensor.matmul(psD[0][:, sl], lhsT=tones[:],
                                         rhs=tile[:, sl],
                                         start=first, stop=last)

                def emit_pv(j, oj, probs):
                    for s in (0, 1):
                        lo, hi = max(oj, 512 * s), 512 * (s + 1)
                        if lo >= hi:
                            continue
                        last = (j == min(jmax, 8 * hf + 4 * s + 3))
                        nc.tensor.matmul(psO[:, lo:hi],
                                         lhsT=tv[:, j * 128:(j + 1) * 128],
                                         rhs=probs[:, lo:hi],
                                         start=(j == 0), stop=last)

                for j in range(jmax + 1):
                    oj = max(0, 128 * j - QH * hf)
                    diag = (j >= 8 * hf)
                    psS = pspool.tile([128, QH], f32, tag="psS")
                    for s in (0, 1):
                        lo, hi = max(oj, 512 * s), 512 * (s + 1)
                        if lo >= hi:
                            continue
                        in_diag_bank = diag and (oj >= 512 * s) and (oj < hi)
                        nc.tensor.matmul(
                            psS[:, lo:hi],
                            lhsT=rk[:, j * 128:(j + 1) * 128],
                            rhs=rq[:, hf * QH + lo: hf * QH + hi],
                            start=True, stop=not in_diag_bank)
                        if in_diag_bank:
                            nc.tensor.matmul(
                                psS[:, oj:oj + 128], lhsT=tid16[:],
                                rhs=tmask[:], start=False, stop=True,
                                skip_group_check=True)
                    # j<2: exp writes straight into the accumulator tile
                    # (it doubles as the probs tile); j>=2: normal probs
                    # tile + accumulate. h0 accumulation runs on GPSIMD,
                    # h1 on DVE (load balance; both SBUF bf16).
                    if j < 2:
                        probs = acc[j]
                        if oj > 0:
                            nc.vector.memset(probs[:, 0:oj], 0.0)
                    else:
                        probs = ppool.tile([128, QH], bf16, tag="probs")
                    nc.scalar.activation(probs[:, oj:QH], psS[:, oj:QH], Exp)
                    # the j=0/1 probs tiles ARE acc0/acc1 — their PV reads
                    # must be emitted before the first add that mutates the
                    # corresponding accumulator (program order defines data
                    # seen, the tile framework only sequences)
                    while (pending and pending[0][0] < 2
                           and pending[0][0] <= j - 2):
                        emit_pv(*pending.pop(0))
                    if j in pe_js:
                        denom_mm(probs, first=(j == min(pe_js)), last=False)
                    elif j >= 2:
                        a = acc[j % 2]
                        eng = (nc.gpsimd if (hf == 0 and H0_ACC_POOL)
                               else nc.vector)
                        eng.tensor_add(a[:, oj:QH], a[:, oj:QH],
                                       probs[:, oj:QH])
                    pending.append((j, oj, probs))
                    if len(pending) >= PEND_DEPTH:
                        emit_pv(*pending.pop(0))
                    if j == 0 and finish_prev is not None:
                        finish_prev()
                    if j == 1 and hook is not None:
                        hook()

                def finish():
                    """Tail of this half: leftover PV consumers, denominator
                    reduce, reciprocal-normalize, output DMA. Deferred into
                    the NEXT half's j-loop so it never blocks the next
                    half's S matmuls / exps in the engine FIFOs."""
                    while pending:
                        emit_pv(*pending.pop(0))
                    denom_mm(acc[0], first=(not pe_js), last=False)
                    denom_mm(acc[1], first=False, last=True)
                    rec = wpool.tile([128, QH], f32, tag="rec")
                    osb = wpool.tile([128, QH], f32, tag="osb")
                    nc.vector.reciprocal_approx_fast(out=rec[:], in_=psD[0][:])
                    nc.vector.tensor_mul(osb[:], psO[:], rec[:])
                    nc.sync.dma_start(out=outT[u, :, hf * QH:(hf + 1) * QH],
                                      in_=osb[:])
                return finish

            # linear schedule over reps*UNITS steps: DMA loads run two units
            # ahead, rope for unit s+1 is emitted inside unit s's h0 (so its
            # DMA has landed long before and the Pool-side rope adds have a
            # full unit of slack before the results are consumed)
            total = reps * UNITS
            raws = {0: load(0)}
            if total > 1:
                raws[1] = load(1 % UNITS)
            cur = rope(0, raws.pop(0))
            fin = None
            for step in range(total):
                u = step % UNITS
                if step + 2 < total:
                    raws[step + 2] = load((step + 2) % UNITS)
                holder = {}
                hook = None
                if step + 1 < total:
                    def hook(step=step, holder=holder):
                        holder["r"] = rope((step + 1) % UNITS,
                                           raws.pop(step + 1))
                fin = attention_half(u, 0, cur[0], cur[1], cur[2],
                                     finish_prev=fin)
                fin = attention_half(u, 1, cur[0], cur[1], cur[2],
                                     hook=hook, finish_prev=fin)
                if step + 1 < total:
                    cur = holder["r"]
            fin()
    nc.compile()
    return nc


def _make_runner(nc):
    """Cached PJRT runner (clone of bass2jax.run_bass_via_pjrt multi-core
    path, but keeping the jitted callable so repeat calls don't recompile)."""
    import jax
    import concourse.mybir as mybir
    from concourse import bass2jax
    from jax.sharding import Mesh, PartitionSpec
    from jax.experimental.shard_map import shard_map

    bass2jax.install_neuronx_cc_hook()

    partition_name = (nc.partition_id_tensor.name
                      if nc.partition_id_tensor else None)
    in_names, out_names, out_avals, zero_outs = [], [], [], []
    for alloc in nc.m.functions[0].allocations:
        if not isinstance(alloc, mybir.MemoryLocationSet):
            continue
        name = alloc.memorylocations[0].name
        if alloc.kind == "ExternalInput":
            if name != partition_name:
                in_names.append(name)
        elif alloc.kind == "ExternalOutput":
            shape = tuple(alloc.tensor_shape)
            dtype = mybir.dt.np(alloc.dtype)
            out_names.append(name)
            out_avals.append(jax.core.ShapedArray(shape, dtype))
            zero_outs.append(np.zeros(shape, dtype))
    n_params = len(in_names)
    n_outs = len(out_avals)
    all_names = in_names + out_names
    if partition_name is not None:
        all_names = all_names + [partition_name]
    donate = tuple(range(n_params, n_params + n_outs))

    def _body(*args):
        operands = list(args)
        if partition_name is not None:
            operands.append(bass2jax.partition_id_tensor())
        outs = bass2jax._bass_exec_p.bind(
            *operands, out_avals=tuple(out_avals), in_names=tuple(all_names),
            out_names=tuple(out_names), lowering_input_output_aliases=(),
            sim_require_finite=True, sim_require_nnan=True, nc=nc)
        return tuple(outs)

    devices = jax.devices()[:N_CORES]
    mesh = Mesh(np.asarray(devices), ("core",))
    sharded = jax.jit(
        shard_map(_body, mesh=mesh,
                  in_specs=(PartitionSpec("core"),) * (n_params + n_outs),
                  out_specs=(PartitionSpec("core"),) * n_outs,
                  check_rep=False),
        donate_argnums=donate, keep_unused=True)

    def run(in_maps):
        concat_in = [np.concatenate([m[name] for m in in_maps], axis=0)
                     for name in in_names]
        concat_zero = [np.concatenate([z] * N_CORES, axis=0) for z in zero_outs]
        outs = sharded(*concat_in, *concat_zero)
        outs = [np.asarray(o) for o in outs]
        res = []
        for c in range(N_CORES):
            d = {}
            for i, name in enumerate(out_names):
                per = outs[i].shape[0] // N_CORES
                d[name] = outs[i][c * per:(c + 1) * per]
            res.append(d)
        return res

    return run


def _rope_tables(start_index):
    half = DH // 2
    inv_freq = np.exp(np.arange(half, dtype=np.float64) *
                      (-(np.log(ROPE_BASE) / half)))
    pos = np.arange(T, dtype=np.float64) + float(start_index)
    ang = pos[:, None] * inv_freq[None, :]          # (T, 64)
    cos = np.cos(ang)
    sin = np.sin(ang)
    cosfull = np.concatenate([cos, cos], axis=1)    # (T, 128) [pos, d]
    sinfull = np.concatenate([-sin, sin], axis=1)
    # [d, pos] layout split per chunk: (2, 128, 2048)
    def lay(x):
        x = x.T.reshape(DH, 2, CHUNK).transpose(1, 0, 2)
        return np.ascontiguousarray(x, dtype=np.float16)
    return lay(cosfull), lay(sinfull)


def _shard_inputs(q, k, v, start_index):
    q = np.asarray(q, dtype=np.float32)
    k = np.asarray(k, dtype=np.float32)
    v = np.asarray(v, dtype=np.float32)
    cosf, sinf = _rope_tables(start_index)
    ident = np.eye(128, dtype=np.float32)
    i = np.arange(128)
    mask16 = np.where(i[:, None] <= i[None, :], 0.0, NEG).astype(ml_dtypes.bfloat16)

    # v layout per unit: [p, blk*128+d] with key pos = blk*128 + p
    def layv(x):  # (2048, 128) -> (128, 2048)
        return x.reshape(NB, 128, DV).transpose(1, 0, 2).reshape(128, CHUNK)

    in_maps = []
    for c in range(N_CORES):
        qu = np.empty((UNITS, 128, CHUNK), np.float16)
        qsu = np.empty((UNITS, 128, CHUNK), np.float16)
        ku = np.empty((UNITS, 128, CHUNK), np.float16)
        ksu = np.empty((UNITS, 128, CHUNK), np.float16)
        vu = np.empty((UNITS, 128, CHUNK), ml_dtypes.bfloat16)
        for ubh in range(BH_PER_CORE):
            bh = c * BH_PER_CORE + ubh
            b, h = bh // H, bh % H
            for ch in range(2):
                u = ubh * 2 + ch
                sl = slice(ch * CHUNK, (ch + 1) * CHUNK)
                qT = q[b, sl, h, :].T.astype(np.float16)   # [d, pos]
                kT = k[b, sl, h, :].T.astype(np.float16)
                qu[u] = qT
                ku[u] = kT
                qsu[u] = np.roll(qT, -64, axis=0)          # row d -> d+64 mod 128
                ksu[u] = np.roll(kT, -64, axis=0)
                vu[u] = layv(v[b, sl, h, :]).astype(ml_dtypes.bfloat16)
        in_maps.append({"qc": qu, "qs": qsu, "kc": ku, "ks": ksu, "vc": vu,
                        "cosf": cosf, "sinf": sinf,
                        "ident16": ident.astype(ml_dtypes.bfloat16),
                        "mask16": mask16})
    return in_maps


def _gather_output(results):
    out = np.empty((B, T, H, DV), np.float32)
    for c in range(N_CORES):
        oT = results[c]["outT"]        # (UNITS, 128 dv, 2048 q)
        for ubh in range(BH_PER_CORE):
            bh = c * BH_PER_CORE + ubh
            b, h = bh // H, bh % H
            for ch in range(2):
                u = ubh * 2 + ch
                out[b, ch * CHUNK:(ch + 1) * CHUNK, h, :] = oT[u].T
    return out


def get_runtime(reps=1):
    if reps not in _RUNTIME:
        nc = _build_program(reps)
        _RUNTIME[reps] = _make_runner(nc)
    return _RUNTIME[reps]


def kernel(q, k, v, start_index):
    run = get_runtime()
    in_maps = _shard_inputs(q, k, v, start_index)
    results = run(in_maps)
    return _gather_output(results)


if __name__ == "__main__":
    rng = np.random.default_rng(0)
    q = rng.standard_normal((B, T, H, DH)).astype(np.float32)
    k = rng.standard_normal((B, T, H, DH)).astype(np.float32)
    v = rng.standard_normal((B, T, H, DV)).astype(np.float32)
    out = kernel(q, k, v, 0)
    print("out", out.shape, out.dtype, np.abs(out).max())


# revision 5
# speedup vs baseline: 1.6570x; 1.0742x over previous
# Chunked causal self-attention (Megalodon-style, chunk=2048) on 8 Trainium2
# NeuronCores via Bass/Tile — v2.
#
# Problem (hardcoded): q,k,v (2, 4096, 16, 128) fp32, RoPE(10000) on q,k,
# per-chunk causal softmax(QK^T)V with scale 1.0.
#
# Sharding: 64 independent (batch, chunk, head) attention units of size
# (2048 x 2048 x 128); 8 units per core (4 (b,h) pairs x 2 chunks).
#
# v2 design (vs v1): engine-balanced around the two irreducible loads —
# PE matmul (S + PV, 1 cyc/col) and ACT exp (only engine with Exp).
#   * q,k arrive fp16 in [d, pos] layout PLUS partition-swapped copies
#     (rows rotated by 64), so RoPE runs on DVE lane-aligned in fp16:
#     rq = q*cosF + qswap*sinF  (3 ops, 2-byte datapath)
#     -> NO PE transposes, NO PSUM->SBUF copies on ACT.
#   * S^T = K^T.T @ Q^T directly from the fp16 rope outputs (fp16 matmul,
#     1 cyc/col, exact-enough: rel err ~0.013 vs 2e-2 gate).
#   * softmax denominators: DVE accumulates probs tiles in bf16 into two
#     interleaved accumulators; ONE final ones-matmul pair per half
#     reduces over partitions (kills the per-j ones-matmul that used to
#     cost PE as much as PV).
#   * mask on diagonal 128-blocks stays a bf16 matmul (cheap, proven).
#   * exp on ACT -> probs bf16; reciprocal+normalize on DVE; DMA out O^T.
import numpy as np
import ml_dtypes

B, T, H, DH, DV = 2, 4096, 16, 128, 128
CHUNK = 2048
NB = CHUNK // 128          # 16 key blocks per chunk
N_CORES = 8
UNITS = 8                  # (b,h) pairs per core * 2 chunks
BH_PER_CORE = (B * H) // N_CORES   # 4
ROPE_BASE = 10000.0
NEG = -1e30
QH = 1024                  # q-half width processed per pass (PSUM budget)

_RUNTIME = {}

import os as _os
ROPE_ADD_POOL = int(_os.environ.get("ROPE_ADD_POOL", "0"))
H0_ACC_POOL = int(_os.environ.get("H0_ACC_POOL", "0"))
PEND_DEPTH = int(_os.environ.get("PEND_DEPTH", "4"))
PE_DENOM_N = int(_os.environ.get("PE_DENOM_N", "0"))
WIDE_MM = int(_os.environ.get("WIDE_MM", "0"))
PV_PAIR = int(_os.environ.get("PV_PAIR", "0"))


def _build_program(reps=1):
    import concourse.tile as tile
    import concourse.mybir as mybir
    from concourse import bacc

    f32 = mybir.dt.float32
    f16 = mybir.dt.float16
    bf16 = mybir.dt.bfloat16
    Exp = mybir.ActivationFunctionType.Exp

    nc = bacc.Bacc("TRN2", target_bir_lowering=False, debug=False,
                   num_devices=N_CORES)

    qc = nc.dram_tensor("qc", [UNITS, 128, CHUNK], f16, kind="ExternalInput").ap()
    qs = nc.dram_tensor("qs", [UNITS, 128, CHUNK], f16, kind="ExternalInput").ap()
    kc = nc.dram_tensor("kc", [UNITS, 128, CHUNK], f16, kind="ExternalInput").ap()
    ks = nc.dram_tensor("ks", [UNITS, 128, CHUNK], f16, kind="ExternalInput").ap()
    vc = nc.dram_tensor("vc", [UNITS, 128, CHUNK], bf16, kind="ExternalInput").ap()
    cosf = nc.dram_tensor("cosf", [2, 128, CHUNK], f16, kind="ExternalInput").ap()
    sinf = nc.dram_tensor("sinf", [2, 128, CHUNK], f16, kind="ExternalInput").ap()
    ident16 = nc.dram_tensor("ident16", [128, 128], bf16, kind="ExternalInput").ap()
    mask16 = nc.dram_tensor("mask16", [128, 128], bf16, kind="ExternalInput").ap()
    outT = nc.dram_tensor("outT", [UNITS, 128, CHUNK], f32, kind="ExternalOutput").ap()

    with tile.TileContext(nc) as tc:
        with tc.tile_pool(name="const", bufs=1) as cpool, \
             tc.tile_pool(name="raw", bufs=3) as rpool, \
             tc.tile_pool(name="rope", bufs=2) as qpool, \
             tc.tile_pool(name="work", bufs=2) as wpool, \
             tc.tile_pool(name="accs", bufs=2) as apool, \
             tc.tile_pool(name="probs", bufs=8) as ppool, \
             tc.tile_pool(name="psum", bufs=2, space="PSUM") as pspool, \
             tc.tile_pool(name="psumO", bufs=1, space="PSUM") as popool, \
             tc.tile_pool(name="psumD", bufs=1, space="PSUM") as pdpool:

            tcos = cpool.tile([128, 2 * CHUNK], f16, tag="tcos")
            tsin = cpool.tile([128, 2 * CHUNK], f16, tag="tsin")
            tid16 = cpool.tile([128, 128], bf16, tag="tid16")
            tmask = cpool.tile([128, 128], bf16, tag="tmask")
            tones = cpool.tile([128, 128], bf16, tag="tones")
            for ch in range(2):
                nc.gpsimd.dma_start(out=tcos[:, ch * CHUNK:(ch + 1) * CHUNK], in_=cosf[ch])
                nc.gpsimd.dma_start(out=tsin[:, ch * CHUNK:(ch + 1) * CHUNK], in_=sinf[ch])
            nc.gpsimd.dma_start(out=tid16[:], in_=ident16[:])
            nc.gpsimd.dma_start(out=tmask[:], in_=mask16[:])
            nc.gpsimd.memset(tones[:], 1.0)

            def load(u):
                """DMA q,k (+swapped) and v of unit u. Returns raw tiles."""
                tq = rpool.tile([128, CHUNK], f16, tag="tq")
                tqs = rpool.tile([128, CHUNK], f16, tag="tqs")
                tk = rpool.tile([128, CHUNK], f16, tag="tk")
                tks = rpool.tile([128, CHUNK], f16, tag="tks")
                tv = rpool.tile([128, CHUNK], bf16, tag="tv")
                nc.sync.dma_start(out=tq[:], in_=qc[u])
                nc.sync.dma_start(out=tqs[:], in_=qs[u])
                nc.sync.dma_start(out=tk[:], in_=kc[u])
                nc.sync.dma_start(out=tks[:], in_=ks[u])
                nc.sync.dma_start(out=tv[:], in_=vc[u])
                return tq, tqs, tk, tks, tv

            def rope(u, raw):
                """RoPE in fp16: r = x*cosF + xswap*sinF. Muls on DVE, the
                final adds on GPSIMD (SBUF-only, Pool is otherwise idle)."""
                tq, tqs, tk, tks, tv = raw
                ch = u % 2
                cosv = tcos[:, ch * CHUNK:(ch + 1) * CHUNK]
                sinv = tsin[:, ch * CHUNK:(ch + 1) * CHUNK]
                outs = []
                for x, xs, tag in ((tq, tqs, "rq"), (tk, tks, "rk")):
                    r = qpool.tile([128, CHUNK], f16, tag=tag)
                    nc.vector.tensor_mul(r[:], x[:], cosv)
                    nc.vector.tensor_mul(xs[:], xs[:], sinv)
                    eng = nc.gpsimd if ROPE_ADD_POOL else nc.vector
                    eng.tensor_add(r[:], r[:], xs[:])
                    outs.append(r)
                return outs[0], outs[1], tv

            def attention_half(u, hf, rq, rk, tv, hook=None,
                               finish_prev=None):
                jmax = 8 * hf + 7
                psO = popool.tile([128, QH], f32, tag="psO")
                acc = [apool.tile([128, QH], bf16, tag=f"acc{i}",
                                  name=f"acc{i}") for i in (0, 1)]
                # h1 denominator js handled directly on PE (ones-matmul into
                # the psD accumulation group) instead of DVE adds
                pe_js = set(range(2, 2 + PE_DENOM_N)) if hf == 1 else set()
                psD = [None]
                pending = []

                def denom_mm(tile, first, last):
                    if psD[0] is None:
                        psD[0] = pdpool.tile([128, QH], f32, tag="psD",
                                             name="psD")
                    if WIDE_MM:
                        nc.tensor.matmul(psD[0][:], lhsT=tones[:],
                                         rhs=tile[:], start=first, stop=last)
                        return
                    for s in (0, 1):
                        sl = slice(512 * s, 512 * (s + 1))
                        nc.tensor.matmul(psD[0][:, sl], lhsT=tones[:],
                                         rhs=tile[:, sl],
                                         start=first, stop=last)

                def emit_pv(j, oj, probs):
                    if WIDE_MM:
                        nc.tensor.matmul(psO[:, oj:QH],
                                         lhsT=tv[:, j * 128:(j + 1) * 128],
                                         rhs=probs[:, oj:QH],
                                         start=(j == 0), stop=(j == jmax),
                                         skip_group_check=True)
                        return
                    for s in (0, 1):
                        lo, hi = max(oj, 512 * s), 512 * (s + 1)
                        if lo >= hi:
                            continue
                        last = (j == min(jmax, 8 * hf + 4 * s + 3))
                        nc.tensor.matmul(psO[:, lo:hi],
                                         lhsT=tv[:, j * 128:(j + 1) * 128],
                                         rhs=probs[:, lo:hi],
                                         start=(j == 0), stop=last)

                for j in range(jmax + 1):
                    oj = max(0, 128 * j - QH * hf)
                    diag = (j >= 8 * hf)
                    psS = pspool.tile([128, QH], f32, tag="psS")
                    if WIDE_MM:
                        nc.tensor.matmul(
                            psS[:, oj:QH],
                            lhsT=rk[:, j * 128:(j + 1) * 128],
                            rhs=rq[:, hf * QH + oj: hf * QH + QH],
                            start=True, stop=True)
                        if diag:
                            nc.tensor.matmul(
                                psS[:, oj:oj + 128], lhsT=tid16[:],
                                rhs=tmask[:], start=False, stop=True,
                                skip_group_check=True)
                    else:
                        for s in (0, 1):
                            lo, hi = max(oj, 512 * s), 512 * (s + 1)
                            if lo >= hi:
                                continue
                            in_diag_bank = (diag and (oj >= 512 * s)
                                            and (oj < hi))
                            nc.tensor.matmul(
                                psS[:, lo:hi],
                                lhsT=rk[:, j * 128:(j + 1) * 128],
                                rhs=rq[:, hf * QH + lo: hf * QH + hi],
                                start=True, stop=not in_diag_bank)
                            if in_diag_bank:
                                nc.tensor.matmul(
                                    psS[:, oj:oj + 128], lhsT=tid16[:],
                                    rhs=tmask[:], start=False, stop=True,
                                    skip_group_check=True)
                    # j<2: exp writes straight into the accumulator tile
                    # (it doubles as the probs tile); j>=2: normal probs
                    # tile + accumulate. h0 accumulation runs on GPSIMD,
                    # h1 on DVE (load balance; both SBUF bf16).
                    if j < 2:
                        probs = acc[j]
                        if oj > 0:
                            nc.vector.memset(probs[:, 0:oj], 0.0)
                    else:
                        probs = ppool.tile([128, QH], bf16, tag="probs")
                    nc.scalar.activation(probs[:, oj:QH], psS[:, oj:QH], Exp)
                    # the j=0/1 probs tiles ARE acc0/acc1 — their PV reads
                    # must be emitted before the first add that mutates the
                    # corresponding accumulator (program order defines data
                    # seen, the tile framework only sequences)
                    while (pending and pending[0][0] < 2
                           and pending[0][0] <= j - 2):
                        emit_pv(*pending.pop(0))
                    if j in pe_js:
                        denom_mm(probs, first=(j == min(pe_js)), last=False)
                    elif j >= 2:
                        a = acc[j % 2]
                        eng = (nc.gpsimd if (hf == 0 and H0_ACC_POOL)
                               else nc.vector)
                        eng.tensor_add(a[:, oj:QH], a[:, oj:QH],
                                       probs[:, oj:QH])
                    pending.append((j, oj, probs))
                    if PV_PAIR:
                        if j % 2 == 1:
                            while len(pending) > PEND_DEPTH - 2:
                                emit_pv(*pending.pop(0))
                    elif len(pending) >= PEND_DEPTH:
                        emit_pv(*pending.pop(0))
                    if j == 0 and finish_prev is not None:
                        finish_prev()
                    if j == 1 and hook is not None:
                        hook()

                def finish():
                    """Tail of this half: leftover PV consumers, denominator
                    reduce, reciprocal-normalize, output DMA. Deferred into
                    the NEXT half's j-loop so it never blocks the next
                    half's S matmuls / exps in the engine FIFOs."""
                    while pending:
                        emit_pv(*pending.pop(0))
                    denom_mm(acc[0], first=(not pe_js), last=False)
                    denom_mm(acc[1], first=False, last=True)
                    rec = wpool.tile([128, QH], f32, tag="rec")
                    osb = wpool.tile([128, QH], f32, tag="osb")
                    nc.vector.reciprocal_approx_fast(out=rec[:], in_=psD[0][:])
                    nc.vector.tensor_mul(osb[:], psO[:], rec[:])
                    nc.sync.dma_start(out=outT[u, :, hf * QH:(hf + 1) * QH],
                                      in_=osb[:])
                return finish

            # linear schedule over reps*UNITS steps: DMA loads run two units
            # ahead, rope for unit s+1 is emitted inside unit s's h0 (so its
            # DMA has landed long before and the Pool-side rope adds have a
            # full unit of slack before the results are consumed)
            total = reps * UNITS
            raws = {0: load(0)}
            if total > 1:
                raws[1] = load(1 % UNITS)
            cur = rope(0, raws.pop(0))
            fin = None
            for step in range(total):
                u = step % UNITS
                if step + 2 < total:
                    raws[step + 2] = load((step + 2) % UNITS)
                holder = {}
                hook = None
                if step + 1 < total:
                    def hook(step=step, holder=holder):
                        holder["r"] = rope((step + 1) % UNITS,
                                           raws.pop(step + 1))
                fin = attention_half(u, 0, cur[0], cur[1], cur[2],
                                     finish_prev=fin)
                fin = attention_half(u, 1, cur[0], cur[1], cur[2],
                                     hook=hook, finish_prev=fin)
                if step + 1 < total:
                    cur = holder["r"]
            fin()
    nc.compile()
    return nc


def _make_runner(nc):
    """Cached PJRT runner (clone of bass2jax.run_bass_via_pjrt multi-core
    path, but keeping the jitted callable so repeat calls don't recompile)."""
    import jax
    import concourse.mybir as mybir
    from concourse import bass2jax
    from jax.sharding import Mesh, PartitionSpec
    from jax.experimental.shard_map import shard_map

    bass2jax.install_neuronx_cc_hook()

    partition_name = (nc.partition_id_tensor.name
                      if nc.partition_id_tensor else None)
    in_names, out_names, out_avals, zero_outs = [], [], [], []
    for alloc in nc.m.functions[0].allocations:
        if not isinstance(alloc, mybir.MemoryLocationSet):
            continue
        name = alloc.memorylocations[0].name
        if alloc.kind == "ExternalInput":
            if name != partition_name:
                in_names.append(name)
        elif alloc.kind == "ExternalOutput":
            shape = tuple(alloc.tensor_shape)
            dtype = mybir.dt.np(alloc.dtype)
            out_names.append(name)
            out_avals.append(jax.core.ShapedArray(shape, dtype))
            zero_outs.append(np.zeros(shape, dtype))
    n_params = len(in_names)
    n_outs = len(out_avals)
    all_names = in_names + out_names
    if partition_name is not None:
        all_names = all_names + [partition_name]
    donate = tuple(range(n_params, n_params + n_outs))

    def _body(*args):
        operands = list(args)
        if partition_name is not None:
            operands.append(bass2jax.partition_id_tensor())
        outs = bass2jax._bass_exec_p.bind(
            *operands, out_avals=tuple(out_avals), in_names=tuple(all_names),
            out_names=tuple(out_names), lowering_input_output_aliases=(),
            sim_require_finite=True, sim_require_nnan=True, nc=nc)
        return tuple(outs)

    devices = jax.devices()[:N_CORES]
    mesh = Mesh(np.asarray(devices), ("core",))
    sharded = jax.jit(
        shard_map(_body, mesh=mesh,
                  in_specs=(PartitionSpec("core"),) * (n_params + n_outs),
                  out_specs=(PartitionSpec("core"),) * n_outs,
                  check_rep=False),
        donate_argnums=donate, keep_unused=True)

    def run(in_maps):
        concat_in = [np.concatenate([m[name] for m in in_maps], axis=0)
                     for name in in_names]
        concat_zero = [np.concatenate([z] * N_CORES, axis=0) for z in zero_outs]
        outs = sharded(*concat_in, *concat_zero)
        outs = [np.asarray(o) for o in outs]
        res = []
        for c in range(N_CORES):
            d = {}
            for i, name in enumerate(out_names):
                per = outs[i].shape[0] // N_CORES
                d[name] = outs[i][c * per:(c + 1) * per]
            res.append(d)
        return res

    return run


def _rope_tables(start_index):
    half = DH // 2
    inv_freq = np.exp(np.arange(half, dtype=np.float64) *
                      (-(np.log(ROPE_BASE) / half)))
    pos = np.arange(T, dtype=np.float64) + float(start_index)
    ang = pos[:, None] * inv_freq[None, :]          # (T, 64)
    cos = np.cos(ang)
    sin = np.sin(ang)
    cosfull = np.concatenate([cos, cos], axis=1)    # (T, 128) [pos, d]
    sinfull = np.concatenate([-sin, sin], axis=1)
    # [d, pos] layout split per chunk: (2, 128, 2048)
    def lay(x):
        x = x.T.reshape(DH, 2, CHUNK).transpose(1, 0, 2)
        return np.ascontiguousarray(x, dtype=np.float16)
    return lay(cosfull), lay(sinfull)


def _shard_inputs(q, k, v, start_index):
    q = np.asarray(q, dtype=np.float32)
    k = np.asarray(k, dtype=np.float32)
    v = np.asarray(v, dtype=np.float32)
    cosf, sinf = _rope_tables(start_index)
    ident = np.eye(128, dtype=np.float32)
    i = np.arange(128)
    mask16 = np.where(i[:, None] <= i[None, :], 0.0, NEG).astype(ml_dtypes.bfloat16)

    # v layout per unit: [p, blk*128+d] with key pos = blk*128 + p
    def layv(x):  # (2048, 128) -> (128, 2048)
        return x.reshape(NB, 128, DV).transpose(1, 0, 2).reshape(128, CHUNK)

    in_maps = []
    for c in range(N_CORES):
        qu = np.empty((UNITS, 128, CHUNK), np.float16)
        qsu = np.empty((UNITS, 128, CHUNK), np.float16)
        ku = np.empty((UNITS, 128, CHUNK), np.float16)
        ksu = np.empty((UNITS, 128, CHUNK), np.float16)
        vu = np.empty((UNITS, 128, CHUNK), ml_dtypes.bfloat16)
        for ubh in range(BH_PER_CORE):
            bh = c * BH_PER_CORE + ubh
            b, h = bh // H, bh % H
            for ch in range(2):
                u = ubh * 2 + ch
                sl = slice(ch * CHUNK, (ch + 1) * CHUNK)
                qT = q[b, sl, h, :].T.astype(np.float16)   # [d, pos]
                kT = k[b, sl, h, :].T.astype(np.float16)
                qu[u] = qT
                ku[u] = kT
                qsu[u] = np.roll(qT, -64, axis=0)          # row d -> d+64 mod 128
                ksu[u] = np.roll(kT, -64, axis=0)
                vu[u] = layv(v[b, sl, h, :]).astype(ml_dtypes.bfloat16)
        in_maps.append({"qc": qu, "qs": qsu, "kc": ku, "ks": ksu, "vc": vu,
                        "cosf": cosf, "sinf": sinf,
                        "ident16": ident.astype(ml_dtypes.bfloat16),
                        "mask16": mask16})
    return in_maps


def _gather_output(results):
    out = np.empty((B, T, H, DV), np.float32)
    for c in range(N_CORES):
        oT = results[c]["outT"]        # (UNITS, 128 dv, 2048 q)
        for ubh in range(BH_PER_CORE):
            bh = c * BH_PER_CORE + ubh
            b, h = bh // H, bh % H
            for ch in range(2):
                u = ubh * 2 + ch
                out[b, ch * CHUNK:(ch + 1) * CHUNK, h, :] = oT[u].T
    return out


def get_runtime(reps=1):
    if reps not in _RUNTIME:
        nc = _build_program(reps)
        _RUNTIME[reps] = _make_runner(nc)
    return _RUNTIME[reps]


def kernel(q, k, v, start_index):
    run = get_runtime()
    in_maps = _shard_inputs(q, k, v, start_index)
    results = run(in_maps)
    return _gather_output(results)


if __name__ == "__main__":
    rng = np.random.default_rng(0)
    q = rng.standard_normal((B, T, H, DH)).astype(np.float32)
    k = rng.standard_normal((B, T, H, DH)).astype(np.float32)
    v = rng.standard_normal((B, T, H, DV)).astype(np.float32)
    out = kernel(q, k, v, 0)
    print("out", out.shape, out.dtype, np.abs(out).max())


# revision 6
# speedup vs baseline: 1.6781x; 1.0127x over previous
# Chunked causal self-attention (Megalodon-style, chunk=2048) on 8 Trainium2
# NeuronCores via Bass/Tile — v2.
#
# Problem (hardcoded): q,k,v (2, 4096, 16, 128) fp32, RoPE(10000) on q,k,
# per-chunk causal softmax(QK^T)V with scale 1.0.
#
# Sharding: 64 independent (batch, chunk, head) attention units of size
# (2048 x 2048 x 128); 8 units per core (4 (b,h) pairs x 2 chunks).
#
# v2 design (vs v1): engine-balanced around the two irreducible loads —
# PE matmul (S + PV, 1 cyc/col) and ACT exp (only engine with Exp).
#   * q,k arrive fp16 in [d, pos] layout PLUS partition-swapped copies
#     (rows rotated by 64), so RoPE runs on DVE lane-aligned in fp16:
#     rq = q*cosF + qswap*sinF  (3 ops, 2-byte datapath)
#     -> NO PE transposes, NO PSUM->SBUF copies on ACT.
#   * S^T = K^T.T @ Q^T directly from the fp16 rope outputs (fp16 matmul,
#     1 cyc/col, exact-enough: rel err ~0.013 vs 2e-2 gate).
#   * softmax denominators: DVE accumulates probs tiles in bf16 into two
#     interleaved accumulators; ONE final ones-matmul pair per half
#     reduces over partitions (kills the per-j ones-matmul that used to
#     cost PE as much as PV).
#   * mask on diagonal 128-blocks stays a bf16 matmul (cheap, proven).
#   * exp on ACT -> probs bf16; reciprocal+normalize on DVE; DMA out O^T.
import numpy as np
import ml_dtypes

B, T, H, DH, DV = 2, 4096, 16, 128, 128
CHUNK = 2048
NB = CHUNK // 128          # 16 key blocks per chunk
N_CORES = 8
UNITS = 8                  # (b,h) pairs per core * 2 chunks
BH_PER_CORE = (B * H) // N_CORES   # 4
ROPE_BASE = 10000.0
NEG = -1e30
QH = 1024                  # q-half width processed per pass (PSUM budget)

_RUNTIME = {}

import os as _os
ROPE_ADD_POOL = int(_os.environ.get("ROPE_ADD_POOL", "0"))
H0_ACC_POOL = int(_os.environ.get("H0_ACC_POOL", "0"))
PEND_DEPTH = int(_os.environ.get("PEND_DEPTH", "4"))
PE_DENOM_N = int(_os.environ.get("PE_DENOM_N", "0"))
WIDE_MM = int(_os.environ.get("WIDE_MM", "0"))
PV_PAIR = int(_os.environ.get("PV_PAIR", "0"))
ACC4 = int(_os.environ.get("ACC4", "0"))


def _build_program(reps=1):
    import concourse.tile as tile
    import concourse.mybir as mybir
    from concourse import bacc

    f32 = mybir.dt.float32
    f16 = mybir.dt.float16
    bf16 = mybir.dt.bfloat16
    Exp = mybir.ActivationFunctionType.Exp

    nc = bacc.Bacc("TRN2", target_bir_lowering=False, debug=False,
                   num_devices=N_CORES)

    qc = nc.dram_tensor("qc", [UNITS, 128, CHUNK], f16, kind="ExternalInput").ap()
    qs = nc.dram_tensor("qs", [UNITS, 128, CHUNK], f16, kind="ExternalInput").ap()
    kc = nc.dram_tensor("kc", [UNITS, 128, CHUNK], f16, kind="ExternalInput").ap()
    ks = nc.dram_tensor("ks", [UNITS, 128, CHUNK], f16, kind="ExternalInput").ap()
    vc = nc.dram_tensor("vc", [UNITS, 128, CHUNK], bf16, kind="ExternalInput").ap()
    cosf = nc.dram_tensor("cosf", [2, 128, CHUNK], f16, kind="ExternalInput").ap()
    sinf = nc.dram_tensor("sinf", [2, 128, CHUNK], f16, kind="ExternalInput").ap()
    ident16 = nc.dram_tensor("ident16", [128, 128], bf16, kind="ExternalInput").ap()
    mask16 = nc.dram_tensor("mask16", [128, 128], bf16, kind="ExternalInput").ap()
    outT = nc.dram_tensor("outT", [UNITS, 128, CHUNK], f32, kind="ExternalOutput").ap()

    with tile.TileContext(nc) as tc:
        with tc.tile_pool(name="const", bufs=1) as cpool, \
             tc.tile_pool(name="raw", bufs=3) as rpool, \
             tc.tile_pool(name="rope", bufs=2) as qpool, \
             tc.tile_pool(name="work", bufs=2) as wpool, \
             tc.tile_pool(name="accs", bufs=2) as apool, \
             tc.tile_pool(name="probs", bufs=8) as ppool, \
             tc.tile_pool(name="psum", bufs=2, space="PSUM") as pspool, \
             tc.tile_pool(name="psumO", bufs=1, space="PSUM") as popool, \
             tc.tile_pool(name="psumD", bufs=1, space="PSUM") as pdpool:

            tcos = cpool.tile([128, 2 * CHUNK], f16, tag="tcos")
            tsin = cpool.tile([128, 2 * CHUNK], f16, tag="tsin")
            tid16 = cpool.tile([128, 128], bf16, tag="tid16")
            tmask = cpool.tile([128, 128], bf16, tag="tmask")
            tones = cpool.tile([128, 128], bf16, tag="tones")
            for ch in range(2):
                nc.gpsimd.dma_start(out=tcos[:, ch * CHUNK:(ch + 1) * CHUNK], in_=cosf[ch])
                nc.gpsimd.dma_start(out=tsin[:, ch * CHUNK:(ch + 1) * CHUNK], in_=sinf[ch])
            nc.gpsimd.dma_start(out=tid16[:], in_=ident16[:])
            nc.gpsimd.dma_start(out=tmask[:], in_=mask16[:])
            nc.gpsimd.memset(tones[:], 1.0)

            def load(u):
                """DMA q,k (+swapped) and v of unit u. Returns raw tiles."""
                tq = rpool.tile([128, CHUNK], f16, tag="tq")
                tqs = rpool.tile([128, CHUNK], f16, tag="tqs")
                tk = rpool.tile([128, CHUNK], f16, tag="tk")
                tks = rpool.tile([128, CHUNK], f16, tag="tks")
                tv = rpool.tile([128, CHUNK], bf16, tag="tv")
                nc.sync.dma_start(out=tq[:], in_=qc[u])
                nc.sync.dma_start(out=tqs[:], in_=qs[u])
                nc.sync.dma_start(out=tk[:], in_=kc[u])
                nc.sync.dma_start(out=tks[:], in_=ks[u])
                nc.sync.dma_start(out=tv[:], in_=vc[u])
                return tq, tqs, tk, tks, tv

            def rope(u, raw):
                """RoPE in fp16: r = x*cosF + xswap*sinF. Muls on DVE, the
                final adds on GPSIMD (SBUF-only, Pool is otherwise idle)."""
                tq, tqs, tk, tks, tv = raw
                ch = u % 2
                cosv = tcos[:, ch * CHUNK:(ch + 1) * CHUNK]
                sinv = tsin[:, ch * CHUNK:(ch + 1) * CHUNK]
                outs = []
                for x, xs, tag in ((tq, tqs, "rq"), (tk, tks, "rk")):
                    r = qpool.tile([128, CHUNK], f16, tag=tag)
                    nc.vector.tensor_mul(r[:], x[:], cosv)
                    nc.vector.tensor_mul(xs[:], xs[:], sinv)
                    eng = nc.gpsimd if ROPE_ADD_POOL else nc.vector
                    eng.tensor_add(r[:], r[:], xs[:])
                    outs.append(r)
                return outs[0], outs[1], tv

            def attention_half(u, hf, rq, rk, tv, hook=None,
                               finish_prev=None):
                jmax = 8 * hf + 7
                nacc = 4 if (hf == 1 and ACC4) else 2
                psO = popool.tile([128, QH], f32, tag="psO")
                acc = [apool.tile([128, QH], bf16, tag=f"acc{hf}{i}",
                                  name=f"acc{hf}{i}") for i in range(nacc)]
                # h1 denominator js handled directly on PE (ones-matmul into
                # the psD accumulation group) instead of DVE adds
                pe_js = set(range(2, 2 + PE_DENOM_N)) if hf == 1 else set()
                psD = [None]
                pending = []

                def denom_mm(tile, first, last):
                    if psD[0] is None:
                        psD[0] = pdpool.tile([128, QH], f32, tag="psD",
                                             name="psD")
                    if WIDE_MM:
                        nc.tensor.matmul(psD[0][:], lhsT=tones[:],
                                         rhs=tile[:], start=first, stop=last)
                        return
                    for s in (0, 1):
                        sl = slice(512 * s, 512 * (s + 1))
                        nc.tensor.matmul(psD[0][:, sl], lhsT=tones[:],
                                         rhs=tile[:, sl],
                                         start=first, stop=last)

                def emit_pv(j, oj, probs):
                    if WIDE_MM:
                        nc.tensor.matmul(psO[:, oj:QH],
                                         lhsT=tv[:, j * 128:(j + 1) * 128],
                                         rhs=probs[:, oj:QH],
                                         start=(j == 0), stop=(j == jmax),
                                         skip_group_check=True)
                        return
                    for s in (0, 1):
                        lo, hi = max(oj, 512 * s), 512 * (s + 1)
                        if lo >= hi:
                            continue
                        last = (j == min(jmax, 8 * hf + 4 * s + 3))
                        nc.tensor.matmul(psO[:, lo:hi],
                                         lhsT=tv[:, j * 128:(j + 1) * 128],
                                         rhs=probs[:, lo:hi],
                                         start=(j == 0), stop=last)

                for j in range(jmax + 1):
                    oj = max(0, 128 * j - QH * hf)
                    diag = (j >= 8 * hf)
                    psS = pspool.tile([128, QH], f32, tag="psS")
                    if WIDE_MM:
                        nc.tensor.matmul(
                            psS[:, oj:QH],
                            lhsT=rk[:, j * 128:(j + 1) * 128],
                            rhs=rq[:, hf * QH + oj: hf * QH + QH],
                            start=True, stop=True)
                        if diag:
                            nc.tensor.matmul(
                                psS[:, oj:oj + 128], lhsT=tid16[:],
                                rhs=tmask[:], start=False, stop=True,
                                skip_group_check=True)
                    else:
                        for s in (0, 1):
                            lo, hi = max(oj, 512 * s), 512 * (s + 1)
                            if lo >= hi:
                                continue
                            in_diag_bank = (diag and (oj >= 512 * s)
                                            and (oj < hi))
                            nc.tensor.matmul(
                                psS[:, lo:hi],
                                lhsT=rk[:, j * 128:(j + 1) * 128],
                                rhs=rq[:, hf * QH + lo: hf * QH + hi],
                                start=True, stop=not in_diag_bank)
                            if in_diag_bank:
                                nc.tensor.matmul(
                                    psS[:, oj:oj + 128], lhsT=tid16[:],
                                    rhs=tmask[:], start=False, stop=True,
                                    skip_group_check=True)
                    # j<2: exp writes straight into the accumulator tile
                    # (it doubles as the probs tile); j>=2: normal probs
                    # tile + accumulate. h0 accumulation runs on GPSIMD,
                    # h1 on DVE (load balance; both SBUF bf16).
                    if j < nacc:
                        probs = acc[j]
                        if oj > 0:
                            nc.vector.memset(probs[:, 0:oj], 0.0)
                    else:
                        probs = ppool.tile([128, QH], bf16, tag="probs")
                    nc.scalar.activation(probs[:, oj:QH], psS[:, oj:QH], Exp)
                    # the first nacc probs tiles ARE the accumulators — their
                    # PV reads must be emitted before the first add that
                    # mutates the corresponding accumulator (program order
                    # defines data seen, the tile framework only sequences)
                    while (pending and pending[0][0] < nacc
                           and pending[0][0] <= j - 2):
                        emit_pv(*pending.pop(0))
                    if j in pe_js:
                        denom_mm(probs, first=(j == min(pe_js)), last=False)
                    elif j >= nacc:
                        a = acc[j % nacc]
                        eng = (nc.gpsimd if (hf == 0 and H0_ACC_POOL)
                               else nc.vector)
                        eng.tensor_add(a[:, oj:QH], a[:, oj:QH],
                                       probs[:, oj:QH])
                    pending.append((j, oj, probs))
                    if PV_PAIR:
                        if j % 2 == 1:
                            while len(pending) > PEND_DEPTH - 2:
                                emit_pv(*pending.pop(0))
                    elif len(pending) >= PEND_DEPTH:
                        emit_pv(*pending.pop(0))
                    if j == 0 and finish_prev is not None:
                        finish_prev()
                    if j == 1 and hook is not None:
                        hook()

                def finish():
                    """Tail of this half: leftover PV consumers, denominator
                    reduce, reciprocal-normalize, output DMA. Deferred into
                    the NEXT half's j-loop so it never blocks the next
                    half's S matmuls / exps in the engine FIFOs."""
                    while pending:
                        emit_pv(*pending.pop(0))
                    for i, a in enumerate(acc):
                        denom_mm(a, first=(i == 0 and not pe_js),
                                 last=(i == len(acc) - 1))
                    rec = wpool.tile([128, QH], f32, tag="rec")
                    osb = wpool.tile([128, QH], f32, tag="osb")
                    nc.vector.reciprocal_approx_fast(out=rec[:], in_=psD[0][:])
                    nc.vector.tensor_mul(osb[:], psO[:], rec[:])
                    nc.sync.dma_start(out=outT[u, :, hf * QH:(hf + 1) * QH],
                                      in_=osb[:])
                return finish

            # linear schedule over reps*UNITS steps: DMA loads run two units
            # ahead, rope for unit s+1 is emitted inside unit s's h0 (so its
            # DMA has landed long before and the Pool-side rope adds have a
            # full unit of slack before the results are consumed)
            total = reps * UNITS
            raws = {0: load(0)}
            if total > 1:
                raws[1] = load(1 % UNITS)
            cur = rope(0, raws.pop(0))
            fin = None
            for step in range(total):
                u = step % UNITS
                if step + 2 < total:
                    raws[step + 2] = load((step + 2) % UNITS)
                holder = {}
                hook = None
                if step + 1 < total:
                    def hook(step=step, holder=holder):
                        holder["r"] = rope((step + 1) % UNITS,
                                           raws.pop(step + 1))
                fin = attention_half(u, 0, cur[0], cur[1], cur[2],
                                     finish_prev=fin)
                fin = attention_half(u, 1, cur[0], cur[1], cur[2],
                                     hook=hook, finish_prev=fin)
                if step + 1 < total:
                    cur = holder["r"]
            fin()
    nc.compile()
    return nc


def _make_runner(nc):
    """Cached PJRT runner (clone of bass2jax.run_bass_via_pjrt multi-core
    path, but keeping the jitted callable so repeat calls don't recompile)."""
    import jax
    import concourse.mybir as mybir
    from concourse import bass2jax
    from jax.sharding import Mesh, PartitionSpec
    from jax.experimental.shard_map import shard_map

    bass2jax.install_neuronx_cc_hook()

    partition_name = (nc.partition_id_tensor.name
                      if nc.partition_id_tensor else None)
    in_names, out_names, out_avals, zero_outs = [], [], [], []
    for alloc in nc.m.functions[0].allocations:
        if not isinstance(alloc, mybir.MemoryLocationSet):
            continue
        name = alloc.memorylocations[0].name
        if alloc.kind == "ExternalInput":
            if name != partition_name:
                in_names.append(name)
        elif alloc.kind == "ExternalOutput":
            shape = tuple(alloc.tensor_shape)
            dtype = mybir.dt.np(alloc.dtype)
            out_names.append(name)
            out_avals.append(jax.core.ShapedArray(shape, dtype))
            zero_outs.append(np.zeros(shape, dtype))
    n_params = len(in_names)
    n_outs = len(out_avals)
    all_names = in_names + out_names
    if partition_name is not None:
        all_names = all_names + [partition_name]
    donate = tuple(range(n_params, n_params + n_outs))

    def _body(*args):
        operands = list(args)
        if partition_name is not None:
            operands.append(bass2jax.partition_id_tensor())
        outs = bass2jax._bass_exec_p.bind(
            *operands, out_avals=tuple(out_avals), in_names=tuple(all_names),
            out_names=tuple(out_names), lowering_input_output_aliases=(),
            sim_require_finite=True, sim_require_nnan=True, nc=nc)
        return tuple(outs)

    devices = jax.devices()[:N_CORES]
    mesh = Mesh(np.asarray(devices), ("core",))
    sharded = jax.jit(
        shard_map(_body, mesh=mesh,
                  in_specs=(PartitionSpec("core"),) * (n_params + n_outs),
                  out_specs=(PartitionSpec("core"),) * n_outs,
                  check_rep=False),
        donate_argnums=donate, keep_unused=True)

    def run(in_maps):
        concat_in = [np.concatenate([m[name] for m in in_maps], axis=0)
                     for name in in_names]
        concat_zero = [np.concatenate([z] * N_CORES, axis=0) for z in zero_outs]
        outs = sharded(*concat_in, *concat_zero)
        outs = [np.asarray(o) for o in outs]
        res = []
        for c in range(N_CORES):
            d = {}
            for i, name in enumerate(out_names):
                per = outs[i].shape[0] // N_CORES
                d[name] = outs[i][c * per:(c + 1) * per]
            res.append(d)
        return res

    return run


def _rope_tables(start_index):
    half = DH // 2
    inv_freq = np.exp(np.arange(half, dtype=np.float64) *
                      (-(np.log(ROPE_BASE) / half)))
    pos = np.arange(T, dtype=np.float64) + float(start_index)
    ang = pos[:, None] * inv_freq[None, :]          # (T, 64)
    cos = np.cos(ang)
    sin = np.sin(ang)
    cosfull = np.concatenate([cos, cos], axis=1)    # (T, 128) [pos, d]
    sinfull = np.concatenate([-sin, sin], axis=1)
    # [d, pos] layout split per chunk: (2, 128, 2048)
    def lay(x):
        x = x.T.reshape(DH, 2, CHUNK).transpose(1, 0, 2)
        return np.ascontiguousarray(x, dtype=np.float16)
    return lay(cosfull), lay(sinfull)


def _shard_inputs(q, k, v, start_index):
    q = np.asarray(q, dtype=np.float32)
    k = np.asarray(k, dtype=np.float32)
    v = np.asarray(v, dtype=np.float32)
    cosf, sinf = _rope_tables(start_index)
    ident = np.eye(128, dtype=np.float32)
    i = np.arange(128)
    mask16 = np.where(i[:, None] <= i[None, :], 0.0, NEG).astype(ml_dtypes.bfloat16)

    # v layout per unit: [p, blk*128+d] with key pos = blk*128 + p
    def layv(x):  # (2048, 128) -> (128, 2048)
        return x.reshape(NB, 128, DV).transpose(1, 0, 2).reshape(128, CHUNK)

    in_maps = []
    for c in range(N_CORES):
        qu = np.empty((UNITS, 128, CHUNK), np.float16)
        qsu = np.empty((UNITS, 128, CHUNK), np.float16)
        ku = np.empty((UNITS, 128, CHUNK), np.float16)
        ksu = np.empty((UNITS, 128, CHUNK), np.float16)
        vu = np.empty((UNITS, 128, CHUNK), ml_dtypes.bfloat16)
        for ubh in range(BH_PER_CORE):
            bh = c * BH_PER_CORE + ubh
            b, h = bh // H, bh % H
            for ch in range(2):
                u = ubh * 2 + ch
                sl = slice(ch * CHUNK, (ch + 1) * CHUNK)
                qT = q[b, sl, h, :].T.astype(np.float16)   # [d, pos]
                kT = k[b, sl, h, :].T.astype(np.float16)
                qu[u] = qT
                ku[u] = kT
                qsu[u] = np.roll(qT, -64, axis=0)          # row d -> d+64 mod 128
                ksu[u] = np.roll(kT, -64, axis=0)
                vu[u] = layv(v[b, sl, h, :]).astype(ml_dtypes.bfloat16)
        in_maps.append({"qc": qu, "qs": qsu, "kc": ku, "ks": ksu, "vc": vu,
                        "cosf": cosf, "sinf": sinf,
                        "ident16": ident.astype(ml_dtypes.bfloat16),
                        "mask16": mask16})
    return in_maps


def _gather_output(results):
    out = np.empty((B, T, H, DV), np.float32)
    for c in range(N_CORES):
        oT = results[c]["outT"]        # (UNITS, 128 dv, 2048 q)
        for ubh in range(BH_PER_CORE):
            bh = c * BH_PER_CORE + ubh
            b, h = bh // H, bh % H
            for ch in range(2):
                u = ubh * 2 + ch
                out[b, ch * CHUNK:(ch + 1) * CHUNK, h, :] = oT[u].T
    return out


def get_runtime(reps=1):
    if reps not in _RUNTIME:
        nc = _build_program(reps)
        _RUNTIME[reps] = _make_runner(nc)
    return _RUNTIME[reps]


def kernel(q, k, v, start_index):
    run = get_runtime()
    in_maps = _shard_inputs(q, k, v, start_index)
    results = run(in_maps)
    return _gather_output(results)


if __name__ == "__main__":
    rng = np.random.default_rng(0)
    q = rng.standard_normal((B, T, H, DH)).astype(np.float32)
    k = rng.standard_normal((B, T, H, DH)).astype(np.float32)
    v = rng.standard_normal((B, T, H, DV)).astype(np.float32)
    out = kernel(q, k, v, 0)
    print("out", out.shape, out.dtype, np.abs(out).max())
